# revision 33
# baseline (speedup 1.0000x reference)
"""Trainium2 Bass kernel for nn_NUFFTLayerMultiChannelInitMixed.

Math: the reference's spread->FFT->filter->IFFT->energy pipeline collapses to
an analytic-spectrum bilinear form. The Gaussian spread is deconvolved exactly
by the deconv^2 filter, so with ghat_n(k) ~ e^{-i k x_n} (alias images carry
weight e^{-tau(M-k)^2} ~ 3e-5 -- negligible vs the 2e-2 gate):

  e_i[n] = sum_k G_i(k) [cos(k x_n) C(k) + sin(k x_n) S(k)] + off_i
  C(k) = sum_n cos(k x_n),  S(k) = sum_n sin(k x_n)
  G_i = pref * w * deconv2 * mult_i * p^2  (~1/k^2 decay; K=32 keeps ~2e-4)

Layout: with K=32, BOTH batches pack into one [128, 1024] trig matrix --
row r: batch r//64, kind (r%64)//32 (cos/sin), k = r%32. One K=7 bf16 phase
matmul per 512 cols builds k*t (3-way bf16 split of t, exact in fp32 PSUM)
with the +1/4-turn cos bias riding in the contraction. Range-reduce
(round-to-nearest via +MAGIC, ACT-led and DVE-led halves to balance engines),
one Sin activation per half (bf16 out, accum_out = row sums for free), then
16 matmuls (stationary = 64-row trig chunk, rhs = G*[C;S] [64, 2]) yield
energies directly in [n-part, channel] layout. The constant per-channel
offset rides the two identically-1 trig rows (cos k=0, and sin k=0 via bias).
Sharding: batch-parallel, 2 of 16 batches per core, no collectives.
"""

import numpy as np

try:
    import concourse.bass as bass
except ImportError:
    import sys
    sys.path.insert(0, "/opt/trn_rl_repo")
    import concourse.bass as bass

import concourse.bacc as bacc
import concourse.mybir as mybir
from concourse import tile
from concourse.bass_utils import run_bass_kernel_spmd

F32 = mybir.dt.float32
BF16 = mybir.dt.bfloat16
AF = mybir.ActivationFunctionType
ALU = mybir.AluOpType

M = 2001
L = 2.0 * np.pi
TAU = 12.0 * (L / (2.0 * np.pi * M)) ** 2
K = 32                   # spectral truncation (1/k^2 filter decay)
N = 1024
B_FULL = 16
NCORES = 8
BPC = B_FULL // NCORES   # batches per core, packed into row halves
MAGIC = 12582912.0       # 1.5 * 2^23: (u + MAGIC) - MAGIC = round-to-nearest(u)
PI = float(np.pi)

_RB = np.arange(128) % 64            # within-batch row index
_KROW = _RB % K                      # k value per row
_BIAS = np.where(_RB <= K, 0.25, 0.0)  # cos rows + the sin k=0 offset row


def _bf16(a):
    a32 = np.asarray(a, dtype=np.float32)
    u32 = a32.view(np.uint32).astype(np.uint64)
    return (((u32 + 0x7FFF + ((u32 >> 16) & 1)) & 0xFFFF0000)
            .astype(np.uint32)).view(np.float32)


def _host_constants(shift0, shift1, amp0, amp1):
    """fp64 host-side k-space weights -> cst2 [128, 2]."""
    k = np.arange(K, dtype=np.float64)
    tau = float(TAU)
    p2 = np.exp(-2.0 * tau * k * k)
    deconv2 = (np.pi / tau) * np.exp(2.0 * tau * k * k)
    mult1 = float(amp0) * (4.0 * np.pi) / (k * k + (1.0 * float(shift0)) ** 2)
    mult2 = float(amp1) * (4.0 * np.pi) / (k * k + (0.5 * float(shift1)) ** 2)
    w = np.full(K, 2.0)
    w[0] = 1.0
    Cc = (M / L) * np.sqrt(4.0 * np.pi * tau)
    scale = 1.0 / ((2.0 * np.pi * M / L) * (2.0 * np.pi))
    pref = scale * Cc * Cc / M
    G1 = pref * w * deconv2 * mult1 * p2
    G2 = pref * w * deconv2 * mult2 * p2

    cst2 = np.zeros((128, 2), dtype=np.float64)
    cst2[:, 0] = G1[_KROW]
    cst2[:, 1] = G2[_KROW]

    # Constant offset off_i = G_i[0]*N - sum(G_i) rides the two rows that are
    # identically 1: cos k=0 (rb=0, bf16-representable part) and sin k=0
    # (rb=K, made 1 by its +0.25 bias; carries the residual) -- no separate
    # offset-add instruction and no bf16 precision loss.
    off1 = float(G1[0] * N - G1.sum())
    off2 = float(G2[0] * N - G2.sum())
    for i, off in enumerate((off1, off2)):
        hi = float(_bf16(np.float32(off)))
        cst2[_RB == 0, i] = hi / N
        cst2[_RB == K, i] = (off - hi) / N
    return cst2.astype(np.float32)


def _pack_t(t_rows):
    """[BPC, N] fp32 t values -> [7, 128+N] bf16: the phase stationary
    [7, 128] at cols 0:128 (first in the DMA), then per-batch 3-way split
    rows of t (rows 3b..3b+2) + ones row 6."""
    ext = np.ones((7, 128 + N), dtype=np.float32)
    for b in range(BPC):
        t = t_rows[b]
        th = _bf16(t)
        tm = _bf16(t.astype(np.float64) - th.astype(np.float64))
        tl = _bf16(t.astype(np.float64) - th.astype(np.float64)
                   - tm.astype(np.float64))
        ext[3 * b + 0, 128:] = th
        ext[3 * b + 1, 128:] = tm
        ext[3 * b + 2, 128:] = tl
    kvb = np.zeros((7, 128), dtype=np.float64)
    for b in range(BPC):
        rows = (np.arange(128) // 64) == b
        for j in range(3):
            kvb[3 * b + j, rows] = _KROW[rows]
    kvb[6] = _BIAS
    ext[:, :128] = kvb
    import ml_dtypes
    return ext.astype(ml_dtypes.bfloat16)


def _build_program(debug=False):
    nc = bacc.Bacc(None, target_bir_lowering=False, debug=debug)
    t_in = nc.declare_dram_parameter("t", [7, 128 + N], BF16, isOutput=False)
    cst_in = nc.declare_dram_parameter("cst2", [128, 2], F32, isOutput=False)
    out_t = nc.declare_dram_parameter("out", [128, 16 * BPC], F32, isOutput=True)

    with tile.TileContext(nc) as tc:
        import contextlib
        with contextlib.ExitStack() as ctx:
            pc = ctx.enter_context(tc.tile_pool(name="const", bufs=1))
            wp = ctx.enter_context(tc.tile_pool(name="work", bufs=2))
            sp = ctx.enter_context(tc.tile_pool(name="small", bufs=1))
            ps_u = ctx.enter_context(tc.tile_pool(name="psu", bufs=2, space="PSUM"))
            ps_T = ctx.enter_context(tc.tile_pool(name="psT", bufs=1, space="PSUM"))

            # Dummy Sin on scratch: makes the FIRST ScalarE op a Sin so the
            # compiler resident-set pick contains sin (its sets also contain
            # identity), avoiding a 1.3us mid-pipeline ACT_TABLE_LOAD swap.
            dummy = sp.tile([1, 2], F32, tag="dummy")
            nc.vector.memset(dummy[:], 0.0)
            dummy2 = sp.tile([1, 2], F32, tag="dummy2")
            nc.scalar.activation(dummy2[:], dummy[:], AF.Sin, scale=1.0)
            magicc = pc.tile([128, 1], F32, tag="magic")
            nc.gpsimd.memset(magicc[:], MAGIC)

            t_ext = pc.tile([7, 128 + N], BF16, tag="t")
            nc.sync.dma_start(t_ext[:, 0:640], t_in[:, 0:640])
            nc.sync.dma_start(t_ext[:, 640:], t_in[:, 640:])
            cst2 = pc.tile([128, 2], F32, tag="cst2")
            nc.sync.dma_start(cst2[:], cst_in[:])
            kvb = t_ext[:, 0:128]

            CS = sp.tile([128, N], BF16, tag="CS")
            csum = sp.tile([128, 2], F32, tag="csum")

            u0 = ps_u.tile([128, 512], F32, tag="u")
            nc.tensor.matmul(u0[:], kvb, t_ext[:, 128:640], start=True, stop=True)
            u1 = ps_u.tile([128, 512], F32, tag="u")
            nc.tensor.matmul(u1[:], kvb, t_ext[:, 640:1152], start=True, stop=True)

            # half 0: DVE-led range reduction (lands first; DVE drains it
            # while ACT handles half 1's Identity in parallel)
            rni = wp.tile([128, 512], F32, tag="rni")
            nc.vector.tensor_scalar(rni[:], u0[:], MAGIC, MAGIC,
                                    ALU.add, ALU.subtract)
            r = wp.tile([128, 512], F32, tag="r")
            nc.vector.tensor_sub(r[:], u0[:], rni[:])
            nc.scalar.activation(CS[:, 0:512], r[:], AF.Sin, scale=2.0 * PI,
                                 accum_out=csum[:, 0:1])
            # half 1: ACT-led (Identity reads PSUM, adds MAGIC)
            rniM = wp.tile([128, 512], F32, tag="rniM")
            nc.scalar.activation(rniM[:], u1[:], AF.Identity, bias=magicc[:])
            negr = wp.tile([128, 512], F32, tag="negr")
            nc.vector.scalar_tensor_tensor(negr[:], rniM[:], MAGIC, u1[:],
                                           ALU.subtract, ALU.subtract)
            nc.scalar.activation(CS[:, 512:1024], negr[:], AF.Sin,
                                 scale=-2.0 * PI, accum_out=csum[:, 1:2])

            # UC = cst2 * (csum0 + csum1); only the stt is on the critical path
            UCa = sp.tile([128, 2], F32, tag="UCa")
            nc.gpsimd.tensor_scalar(UCa[:], cst2[:], csum[:, 0:1],
                                    None, ALU.mult)
            UC = sp.tile([128, 2], BF16, tag="UC")
            nc.vector.scalar_tensor_tensor(UC[:], cst2[:], csum[:, 1:2], UCa[:],
                                           ALU.mult, ALU.add)

            pT = ps_T.tile([128, 16 * BPC], F32, tag="pT")
            e = sp.tile([128, 16 * BPC], F32, tag="e")
            for b in range(BPC):
                for j in range(8):
                    lh = CS[64 * b:64 * (b + 1), 128 * j:128 * (j + 1)]
                    nc.tensor.matmul(pT[:, 16 * b + 2 * j: 16 * b + 2 * j + 2],
                                     lh, UC[64 * b:64 * (b + 1), :],
                                     start=True, stop=True)
            for b in range(BPC):
                nc.vector.tensor_copy(e[:, 16 * b:16 * (b + 1)],
                                      pT[:, 16 * b:16 * (b + 1)])
                nc.sync.dma_start(out_t[:, 16 * b:16 * (b + 1)],
                                  e[:, 16 * b:16 * (b + 1)])
    return nc


def kernel(x, shift0, shift1, amp0, amp1):
    x = np.asarray(x, dtype=np.float32)
    cst2 = _host_constants(
        np.asarray(shift0).reshape(-1)[0], np.asarray(shift1).reshape(-1)[0],
        np.asarray(amp0).reshape(-1)[0], np.asarray(amp1).reshape(-1)[0])
    nc = _build_program()
    nc.finalize()

    t_full = (x.astype(np.float64) / (2.0 * np.pi)).astype(np.float32)
    in_maps = []
    for c in range(NCORES):
        t_ext = _pack_t(t_full[BPC * c: BPC * (c + 1)])
        in_maps.append({"t": t_ext, "cst2": cst2})
    res = run_bass_kernel_spmd(nc, in_maps, list(range(NCORES)))
    outs = []
    for c in range(NCORES):
        arr = res.results[c]["out"]                      # [128, 16*BPC]
        arr = arr.reshape(128, BPC, 8, 2)                # (p, b, j, i)
        outs.append(arr.transpose(1, 2, 0, 3).reshape(BPC, N, 2))
    return np.concatenate(outs, axis=0).astype(np.float32)


# revision 34
# speedup vs baseline: 1.0243x; 1.0243x over previous
"""Trainium2 Bass kernel for nn_NUFFTLayerMultiChannelInitMixed.

Math: the reference's spread->FFT->filter->IFFT->energy pipeline collapses to
an analytic-spectrum bilinear form. The Gaussian spread is deconvolved exactly
by the deconv^2 filter, so with ghat_n(k) ~ e^{-i k x_n} (alias images carry
weight e^{-tau(M-k)^2} ~ 3e-5 -- negligible vs the 2e-2 gate):

  e_i[n] = sum_k G_i(k) [cos(k x_n) C(k) + sin(k x_n) S(k)] + off_i
  C(k) = sum_n cos(k x_n),  S(k) = sum_n sin(k x_n)
  G_i = pref * w * deconv2 * mult_i * p^2  (~1/k^2 decay; K=32 keeps ~2e-4)

Layout: with K=32, BOTH batches pack into one [128, 1024] trig matrix --
row r: batch r//64, kind (r%64)//32 (cos/sin), k = r%32. One K=7 bf16 phase
matmul per 512 cols builds k*t (3-way bf16 split of t, exact in fp32 PSUM)
with the +1/4-turn cos bias riding in the contraction. Range-reduce
(round-to-nearest via +MAGIC, ACT-led and DVE-led halves to balance engines),
one Sin activation per half (bf16 out, accum_out = row sums for free), then
16 matmuls (stationary = 64-row trig chunk, rhs = G*[C;S] [64, 2]) yield
energies directly in [n-part, channel] layout. The constant per-channel
offset rides the two identically-1 trig rows (cos k=0, and sin k=0 via bias).
Sharding: batch-parallel, 2 of 16 batches per core, no collectives.
"""

import numpy as np

try:
    import concourse.bass as bass
except ImportError:
    import sys
    sys.path.insert(0, "/opt/trn_rl_repo")
    import concourse.bass as bass

import concourse.bacc as bacc
import concourse.mybir as mybir
from concourse import tile
from concourse.bass_utils import run_bass_kernel_spmd

F32 = mybir.dt.float32
BF16 = mybir.dt.bfloat16
AF = mybir.ActivationFunctionType
ALU = mybir.AluOpType

M = 2001
L = 2.0 * np.pi
TAU = 12.0 * (L / (2.0 * np.pi * M)) ** 2
K = 32                   # spectral truncation (1/k^2 filter decay)
N = 1024
B_FULL = 16
NCORES = 8
BPC = B_FULL // NCORES   # batches per core, packed into row halves
MAGIC = 12582912.0       # 1.5 * 2^23: (u + MAGIC) - MAGIC = round-to-nearest(u)
PI = float(np.pi)

_RB = np.arange(128) % 64            # within-batch row index
_KROW = _RB % K                      # k value per row
_BIAS = np.where(_RB <= K, 0.25, 0.0)  # cos rows + the sin k=0 offset row


def _bf16(a):
    a32 = np.asarray(a, dtype=np.float32)
    u32 = a32.view(np.uint32).astype(np.uint64)
    return (((u32 + 0x7FFF + ((u32 >> 16) & 1)) & 0xFFFF0000)
            .astype(np.uint32)).view(np.float32)


def _host_constants(shift0, shift1, amp0, amp1):
    """fp64 host-side k-space weights -> cst2 [128, 2]."""
    k = np.arange(K, dtype=np.float64)
    tau = float(TAU)
    p2 = np.exp(-2.0 * tau * k * k)
    deconv2 = (np.pi / tau) * np.exp(2.0 * tau * k * k)
    mult1 = float(amp0) * (4.0 * np.pi) / (k * k + (1.0 * float(shift0)) ** 2)
    mult2 = float(amp1) * (4.0 * np.pi) / (k * k + (0.5 * float(shift1)) ** 2)
    w = np.full(K, 2.0)
    w[0] = 1.0
    Cc = (M / L) * np.sqrt(4.0 * np.pi * tau)
    scale = 1.0 / ((2.0 * np.pi * M / L) * (2.0 * np.pi))
    pref = scale * Cc * Cc / M
    G1 = pref * w * deconv2 * mult1 * p2
    G2 = pref * w * deconv2 * mult2 * p2

    cst2 = np.zeros((128, 2), dtype=np.float64)
    cst2[:, 0] = G1[_KROW]
    cst2[:, 1] = G2[_KROW]

    # Constant offset off_i = G_i[0]*N - sum(G_i) rides the two rows that are
    # identically 1: cos k=0 (rb=0, bf16-representable part) and sin k=0
    # (rb=K, made 1 by its +0.25 bias; carries the residual) -- no separate
    # offset-add instruction and no bf16 precision loss.
    off1 = float(G1[0] * N - G1.sum())
    off2 = float(G2[0] * N - G2.sum())
    for i, off in enumerate((off1, off2)):
        hi = float(_bf16(np.float32(off)))
        cst2[_RB == 0, i] = hi / N
        cst2[_RB == K, i] = (off - hi) / N
    return cst2.astype(np.float32)


def _pack_t(t_rows):
    """[BPC, N] fp32 t values -> [7, 128+N] bf16: the phase stationary
    [7, 128] at cols 0:128 (first in the DMA), then per-batch 3-way split
    rows of t (rows 3b..3b+2) + ones row 6."""
    ext = np.ones((7, 128 + N), dtype=np.float32)
    for b in range(BPC):
        t = t_rows[b]
        th = _bf16(t)
        tm = _bf16(t.astype(np.float64) - th.astype(np.float64))
        tl = _bf16(t.astype(np.float64) - th.astype(np.float64)
                   - tm.astype(np.float64))
        ext[3 * b + 0, 128:] = th
        ext[3 * b + 1, 128:] = tm
        ext[3 * b + 2, 128:] = tl
    kvb = np.zeros((7, 128), dtype=np.float64)
    for b in range(BPC):
        rows = (np.arange(128) // 64) == b
        for j in range(3):
            kvb[3 * b + j, rows] = _KROW[rows]
    kvb[6] = _BIAS
    ext[:, :128] = kvb
    import ml_dtypes
    return ext.astype(ml_dtypes.bfloat16)


def _build_program(debug=False):
    nc = bacc.Bacc(None, target_bir_lowering=False, debug=debug)
    t_in = nc.declare_dram_parameter("t", [7, 128 + N], BF16, isOutput=False)
    cst_in = nc.declare_dram_parameter("cst2", [128, 2], F32, isOutput=False)
    out_t = nc.declare_dram_parameter("out", [128, 16 * BPC], F32, isOutput=True)

    with tile.TileContext(nc) as tc:
        import contextlib
        with contextlib.ExitStack() as ctx:
            pc = ctx.enter_context(tc.tile_pool(name="const", bufs=1))
            wp = ctx.enter_context(tc.tile_pool(name="work", bufs=2))
            sp = ctx.enter_context(tc.tile_pool(name="small", bufs=1))
            ps_u = ctx.enter_context(tc.tile_pool(name="psu", bufs=2, space="PSUM"))
            ps_T = ctx.enter_context(tc.tile_pool(name="psT", bufs=1, space="PSUM"))

            # Dummy Sin on scratch: makes the FIRST ScalarE op a Sin so the
            # compiler resident-set pick contains sin (its sets also contain
            # identity), avoiding a 1.3us mid-pipeline ACT_TABLE_LOAD swap.
            dummy = sp.tile([1, 2], F32, tag="dummy")
            nc.vector.memset(dummy[:], 0.0)
            dummy2 = sp.tile([1, 2], F32, tag="dummy2")
            nc.scalar.activation(dummy2[:], dummy[:], AF.Sin, scale=1.0)
            magicc = pc.tile([128, 1], F32, tag="magic")
            nc.gpsimd.memset(magicc[:], MAGIC)

            t_ext = pc.tile([7, 128 + N], BF16, tag="t")
            nc.sync.dma_start(t_ext[:, 0:640], t_in[:, 0:640])
            nc.sync.dma_start(t_ext[:, 640:], t_in[:, 640:])
            cst2 = pc.tile([128, 2], F32, tag="cst2")
            nc.sync.dma_start(cst2[:], cst_in[:])
            kvb = t_ext[:, 0:128]

            CS = sp.tile([128, N], BF16, tag="CS")
            csum = sp.tile([128, 2], F32, tag="csum")

            u0 = ps_u.tile([128, 512], F32, tag="u")
            nc.tensor.matmul(u0[:], kvb, t_ext[:, 128:640], start=True, stop=True)
            u1 = ps_u.tile([128, 512], F32, tag="u")
            nc.tensor.matmul(u1[:], kvb, t_ext[:, 640:1152], start=True, stop=True)

            # half 0: DVE-led range reduction (lands first; DVE drains it
            # while ACT handles half 1's Identity in parallel)
            rni = wp.tile([128, 512], F32, tag="rni")
            nc.vector.tensor_scalar(rni[:], u0[:], MAGIC, MAGIC,
                                    ALU.add, ALU.subtract)
            r = wp.tile([128, 512], F32, tag="r")
            nc.vector.tensor_sub(r[:], u0[:], rni[:])
            nc.scalar.activation(CS[:, 0:512], r[:], AF.Sin, scale=2.0 * PI,
                                 accum_out=csum[:, 0:1])
            # half 1: ACT-led (Identity reads PSUM, adds MAGIC)
            rniM = wp.tile([128, 512], F32, tag="rniM")
            nc.scalar.activation(rniM[:], u1[:], AF.Identity, bias=magicc[:])
            negr = wp.tile([128, 512], F32, tag="negr")
            nc.vector.scalar_tensor_tensor(negr[:], rniM[:], MAGIC, u1[:],
                                           ALU.subtract, ALU.subtract)
            nc.scalar.activation(CS[:, 512:1024], negr[:], AF.Sin,
                                 scale=-2.0 * PI, accum_out=csum[:, 1:2])

            # UC = cst2 * (csum0 + csum1); only the stt is on the critical path
            UCa = sp.tile([128, 2], F32, tag="UCa")
            nc.gpsimd.tensor_scalar(UCa[:], cst2[:], csum[:, 0:1],
                                    None, ALU.mult)
            UC = sp.tile([128, 2], BF16, tag="UC")
            nc.vector.scalar_tensor_tensor(UC[:], cst2[:], csum[:, 1:2], UCa[:],
                                           ALU.mult, ALU.add)

            pT = ps_T.tile([128, 16 * BPC], F32, tag="pT")
            e = sp.tile([128, 16 * BPC], F32, tag="e")
            for b in range(BPC):
                for j in range(8):
                    lh = CS[64 * b:64 * (b + 1), 128 * j:128 * (j + 1)]
                    nc.tensor.matmul(pT[:, 16 * b + 2 * j: 16 * b + 2 * j + 2],
                                     lh, UC[64 * b:64 * (b + 1), :],
                                     start=True, stop=True)
                nc.vector.tensor_copy(e[:, 16 * b:16 * (b + 1)],
                                      pT[:, 16 * b:16 * (b + 1)])
                nc.sync.dma_start(out_t[:, 16 * b:16 * (b + 1)],
                                  e[:, 16 * b:16 * (b + 1)])
    return nc


def kernel(x, shift0, shift1, amp0, amp1):
    x = np.asarray(x, dtype=np.float32)
    cst2 = _host_constants(
        np.asarray(shift0).reshape(-1)[0], np.asarray(shift1).reshape(-1)[0],
        np.asarray(amp0).reshape(-1)[0], np.asarray(amp1).reshape(-1)[0])
    nc = _build_program()
    nc.finalize()

    t_full = (x.astype(np.float64) / (2.0 * np.pi)).astype(np.float32)
    in_maps = []
    for c in range(NCORES):
        t_ext = _pack_t(t_full[BPC * c: BPC * (c + 1)])
        in_maps.append({"t": t_ext, "cst2": cst2})
    res = run_bass_kernel_spmd(nc, in_maps, list(range(NCORES)))
    outs = []
    for c in range(NCORES):
        arr = res.results[c]["out"]                      # [128, 16*BPC]
        arr = arr.reshape(128, BPC, 8, 2)                # (p, b, j, i)
        outs.append(arr.transpose(1, 2, 0, 3).reshape(BPC, N, 2))
    return np.concatenate(outs, axis=0).astype(np.float32)
